# revision 28
# baseline (speedup 1.0000x reference)
"""Trainium2 Bass kernel for the dense_transformer problem.

Data-parallel over batch: 8 NeuronCores x (B/8) sequences each.
Heavy matmuls run in fp8e4 with DoubleRow perf mode (2 K-subtiles per
pass) accumulating in fp32 PSUM. Weights / embeddings / cqc are pre-cast
to fp8 on the host. Emission is software-pipelined at sub-stage
granularity: the attention front-half of sequence bg interleaves with
the MLP/conv back-half of sequence bg-1 so every engine always has
independent work queued.

Self-contained: only imports numpy + installed concourse package.
"""

import os
import numpy as np
from contextlib import ExitStack

import concourse.bass as bass
import concourse.bacc as bacc
import concourse.mybir as mybir
import concourse.tile as tile
from concourse.bass_utils import run_bass_kernel_spmd
from concourse.masks import make_identity, make_upper_triangular

# problem dims (hardcoded per harness contract)
B, L, D, C, NQ, KW, NL = 64, 1024, 256, 256, 10000, 4, 3
NCORES = 8
P = 128
F32 = mybir.dt.float32
BF16 = mybir.dt.bfloat16
F8 = mybir.dt.float8e4
I32 = mybir.dt.int32
AF = mybir.ActivationFunctionType
ALU = mybir.AluOpType
DRM = mybir.MatmulPerfMode.DoubleRow

LT = L // P             # 8 token tiles of 128
NKT = (4 * D + C) // P  # 10 feature tiles of H
PAD = KW - 1            # 3 causal pad cols
XSTR = 1040             # padded free stride for conv input (16-aligned)


def _emit(nc, tc, ctx, dram, nb, repeat=1):
    sb = ctx.enter_context(tc.tile_pool(name="sb", bufs=1))
    seq = ctx.enter_context(tc.tile_pool(name="seq", bufs=1))
    wk = ctx.enter_context(tc.tile_pool(name="wk", bufs=1))
    ps = ctx.enter_context(tc.tile_pool(name="ps", bufs=1, space="PSUM"))

    # ---------------- constants ----------------
    ident8 = sb.tile([P, P], F8, tag="ident8")
    make_identity(nc, ident8[:])
    ident32 = sb.tile([P, P], F32, tag="ident32")
    make_identity(nc, ident32[:])
    smask8 = sb.tile([P, P], F8, tag="smask8")  # strict upper: 1.0 where part < free
    make_upper_triangular(nc, smask8[:], val=1.0, diag=False)
    ones8 = sb.tile([P, 2, 16], F8, tag="ones8")
    nc.gpsimd.memset(ones8[:], 1.0)
    ones16 = sb.tile([P, 1], BF16, tag="ones16")
    nc.gpsimd.memset(ones16[:], 1.0)

    # biases (fp32, per-partition layout)
    w1b = sb.tile([P, 2], F32, tag="w1b")
    w2b = sb.tile([P, 2], F32, tag="w2b")
    for dh in range(2):
        nc.sync.dma_start(out=w1b[:, dh : dh + 1], in_=dram["w1b"][dh * P : (dh + 1) * P, None])
        nc.sync.dma_start(out=w2b[:, dh : dh + 1], in_=dram["w2b"][dh * P : (dh + 1) * P, None])
    convb = sb.tile([P, NL * 4], F32, tag="convb")
    for ly in range(NL):
        for oc in range(4):
            nc.sync.dma_start(
                out=convb[:, ly * 4 + oc : ly * 4 + oc + 1],
                in_=dram["convb"][ly, oc * P : (oc + 1) * P, None],
            )

    # ecpair rows: 0 = ec1-ec0 (diff), 1 = ec0  (bf16, built on host)
    ecpair = sb.tile([2, D], BF16, tag="ecpair")
    nc.sync.dma_start(out=ecpair[:], in_=dram["ecpair"][:])

    # fp8 weights (host pre-transposed/cast): single DMA each
    w1t8 = sb.tile([P, NKT, D], F8, tag="w1t8")
    w2t8 = sb.tile([P, NKT, D], F8, tag="w2t8")
    cw8 = sb.tile([P, NL * KW * 2, 2 * D], F8, tag="cw8")

    def emit_weights():
        nc.sync.dma_start(out=w1t8[:], in_=dram["w1t8"][:])
        nc.sync.dma_start(out=w2t8[:], in_=dram["w2t8"][:])
        nc.sync.dma_start(out=cw8[:], in_=dram["cw8"][:])

    # ---------------- per-sequence pipeline stages ----------------
    issued = {}

    def prep_issue(bg):
        """DMA-only: start gathers + staging loads for sequence bg early."""
        # lis8: [tok_part, lt, qe(256)|ce(256)] fp8
        lis8 = seq.tile([P, LT, 2 * D], F8, tag="lis8", bufs=3, name=f"lis8_{bg}")
        # ht8: [chan_part, kt, tok] fp8; kt 0-1 qeT, 2-3 ceT, 4-7 hrpT, 8-9 cqcT
        ht8 = seq.tile([P, NKT, L], F8, tag="ht8", bufs=3, name=f"ht8_{bg}")
        nc.sync.dma_start(out=lis8[:, :, 0:D], in_=dram["qe8"][bg])
        # cqcT straight into ht8 kt 8-9
        nc.sync.dma_start(out=ht8[:, 8:10, :], in_=dram["cqc8"][bg])
        # corr2 rows: 0 = correctness (cast in prepc), 1 = ones
        corr2 = wk.tile([2, L], BF16, tag="corr2", bufs=2, name=f"corr2_{bg}")
        nc.sync.dma_start(out=corr2[1:2, :], in_=dram["onesrow"][:])
        corr_i = wk.tile([1, L], I32, tag="corr_i", bufs=2, name=f"corri{bg}")
        nc.sync.dma_start(out=corr_i[:], in_=dram["cseq"][bg : bg + 1, :])
        issued[bg] = (lis8, ht8, corr2, corr_i)

    def prepc(bg):
        """Build LIS + HT feature blocks for sequence bg."""
        lis8, ht8, corr2, corr_i = issued.pop(bg)
        nc.vector.tensor_copy(corr2[0:1, :], corr_i[:])

        # ce into lis8 via K=2 matmul: ce = corr*diff + 1*ec0
        for lt in range(LT):
            cep = ps.tile([P, 4 * P], F32, tag="mm", bufs=2, name=f"cep{bg}_{lt}")
            nc.tensor.matmul(
                cep[:, 0:D], lhsT=corr2[0:2, lt * P : (lt + 1) * P],
                rhs=ecpair[0:2, 0:D], start=True, stop=True,
            )
            nc.vector.tensor_copy(lis8[:, lt, D : 2 * D], cep[:, 0:D])

        # ceT into ht8 kt 2-3 via K=2 matmul
        for dh in range(2):
            for lt2 in range(2):
                cetp = ps.tile([P, 4 * P], F32, tag="mm", bufs=2, name=f"cetp{bg}_{dh}_{lt2}")
                nc.tensor.matmul(
                    cetp[:], lhsT=ecpair[0:2, dh * P : (dh + 1) * P],
                    rhs=corr2[0:2, lt2 * 4 * P : (lt2 + 1) * 4 * P],
                    start=True, stop=True,
                )
                nc.vector.tensor_copy(ht8[:, 2 + dh, lt2 * 4 * P : (lt2 + 1) * 4 * P], cetp[:])

        # qeT into ht8 kt 0-1 via PE transpose of lis8 qe cols
        for lt in range(LT):
            tp = ps.tile([P, 2, 2 * P], F8, tag="small", bufs=2, name=f"qtp{bg}_{lt}")
            for dh in range(2):
                nc.tensor.transpose(
                    out=tp[:, dh, 0 : 2 * P : 2], in_=lis8[:, lt, dh * P : (dh + 1) * P],
                    identity=ident8[:],
                )
            nc.vector.tensor_copy(ht8[:, 0:2, lt * P : (lt + 1) * P], tp[:, 0:2, 0 : 2 * P : 2])
        return lis8, ht8

    def attn_mm(bg, iw, lis8, ht8):
        """Scores+exp for i-half iw, then HRP accumulation; defer transposes."""
        jmax = iw * 4 + 4
        # tw8: [j_part, jb, i-col] fp8 exp-scores for this i-half
        tw8 = wk.tile([P, LT, 4 * P], F8, tag="tw8", bufs=3, name=f"tw{bg}_{iw}")
        for jb in range(jmax):
            # cols below the diagonal block are never read (jb <= ib)
            rel = max(jb * P - iw * 4 * P, 0)
            n_live = 4 * P - rel
            scp = ps.tile([P, 4 * P], F32, tag="mm", bufs=2, name=f"scp{bg}_{iw}_{jb}")
            nc.tensor.matmul(
                scp[:, 0:n_live],
                lhsT=ht8[:, 0:2, jb * P : (jb + 1) * P],
                rhs=ht8[:, 0:2, iw * 4 * P + rel : (iw + 1) * 4 * P],
                start=True, stop=True, perf_mode=DRM,
            )
            nc.scalar.activation(tw8[:, jb, rel : 4 * P], scp[:, 0:n_live], AF.Exp)
            if jb * P - iw * 4 * P >= 0:
                nc.vector.tensor_mul(
                    tw8[:, jb, rel : rel + P], tw8[:, jb, rel : rel + P], smask8[:]
                )
        # row sums s[1, i] via ones-lhsT accumulation (1-col weight load),
        # then one reciprocal on the row; per-ib transposes give the
        # per-partition scale the Act engine needs.
        srow_ps = ps.tile([P, 4 * P], F32, tag="mm", bufs=2, name=f"srow{bg}_{iw}")
        for jb in range(jmax):
            rel = max(jb * P - iw * 4 * P, 0)
            nc.tensor.matmul(
                srow_ps[0:1, rel : 4 * P], lhsT=ones8[:, 0, 0:1],
                rhs=tw8[:, jb, rel : 4 * P],
                start=(jb == 0), stop=(jb == jmax - 1),
            )
        srow = wk.tile([1, 4 * P], F32, tag="srow", bufs=2, name=f"srowb{bg}_{iw}")
        nc.vector.tensor_scalar_add(srow[:], srow_ps[0:1, :], 1e-8)
        nc.vector.reciprocal(srow[:], srow[:])
        defer = []
        for isub in range(4):
            ib = iw * 4 + isub
            njb = ib + 1
            hrp_ps = ps.tile([P, 2 * D], F32, tag="mm", bufs=2, name=f"hrpp{bg}_{ib}")
            npair = njb // 2
            for jp in range(npair):
                js = 2 * jp
                first, last = jp == 0, (jp == npair - 1 and njb % 2 == 0)
                tsl = tw8[:, js : js + 2, isub * P : (isub + 1) * P]
                nc.tensor.matmul(
                    hrp_ps[:], lhsT=tsl, rhs=lis8[:, js : js + 2, :],
                    start=first, stop=last, perf_mode=DRM,
                )
            if njb % 2 == 1:
                tsl = tw8[:, ib, isub * P : (isub + 1) * P]
                nc.tensor.matmul(
                    hrp_ps[:], lhsT=tsl, rhs=lis8[:, ib, :],
                    start=(njb == 1), stop=True,
                )
            sp = wk.tile([P, 1], F32, tag="sp", bufs=3, name=f"sp{bg}_{ib}")
            spp = ps.tile([P, 4 * P], F32, tag="mm", bufs=2, name=f"spp{bg}_{ib}")
            nc.tensor.matmul(
                spp[:, 0:1], lhsT=srow[0:1, isub * P : (isub + 1) * P],
                rhs=ident32[0:1, 0:1], start=True, stop=True,
            )
            nc.vector.tensor_copy(sp[:], spp[:, 0:1])
            hrp8 = wk.tile([P, 2 * D], F8, tag="hrp8", bufs=3, name=f"hrp8{bg}_{ib}")
            nc.scalar.activation(hrp8[:], hrp_ps[:], AF.Identity, scale=sp[:, 0:1])
            defer.append((ib, hrp8))
        return defer

    def attn_tp(bg, ht8, defer):
        """Deferred hrpT transposes into ht8 kt 4-7."""
        for ib, hrp8 in defer:
            tp = ps.tile([P, 4, 2 * P], F8, tag="small", bufs=2, name=f"htp{bg}_{ib}")
            for dh in range(4):
                nc.tensor.transpose(
                    out=tp[:, dh, 0 : 2 * P : 2], in_=hrp8[:, dh * P : (dh + 1) * P],
                    identity=ident8[:],
                )
            nc.vector.tensor_copy(
                ht8[:, 4:8, ib * P : (ib + 1) * P], tp[:, 0:4, 0 : 2 * P : 2]
            )

    def mlp(bg, ht8):
        """H @ W1 * sigmoid(H @ W2) -> fp8 conv input xcur."""
        xcur = seq.tile([P, 2, XSTR], F8, tag="xbuf", bufs=5, name=f"xq{bg}")
        nc.gpsimd.memset(xcur[:, :, 0:PAD], 0.0)
        for dh in range(2):
            p1 = ps.tile([P, 2, 4 * P], F32, tag="mm2", bufs=2, name=f"p1_{bg}_{dh}")
            p2 = ps.tile([P, 2, 4 * P], F32, tag="mm2", bufs=2, name=f"p2_{bg}_{dh}")
            for lt2 in range(2):
                for kp in range(NKT // 2):
                    nc.tensor.matmul(
                        p2[:, lt2, :], lhsT=w2t8[:, 2 * kp : 2 * kp + 2, dh * P : (dh + 1) * P],
                        rhs=ht8[:, 2 * kp : 2 * kp + 2, lt2 * 4 * P : (lt2 + 1) * 4 * P],
                        start=(kp == 0), stop=(kp == NKT // 2 - 1), perf_mode=DRM,
                    )
                for kp in range(NKT // 2):
                    nc.tensor.matmul(
                        p1[:, lt2, :], lhsT=w1t8[:, 2 * kp : 2 * kp + 2, dh * P : (dh + 1) * P],
                        rhs=ht8[:, 2 * kp : 2 * kp + 2, lt2 * 4 * P : (lt2 + 1) * 4 * P],
                        start=(kp == 0), stop=(kp == NKT // 2 - 1), perf_mode=DRM,
                    )
            gate = wk.tile([P, 2 * 4 * P], BF16, tag="gate", bufs=4, name=f"gmlp{bg}_{dh}")
            nc.scalar.activation(gate[:], p2[:], AF.Sigmoid, bias=w2b[:, dh : dh + 1])
            nc.vector.scalar_tensor_tensor(
                out=xcur[:, dh, PAD : PAD + L],
                in0=p1[:], scalar=w1b[:, dh : dh + 1], in1=gate[:],
                op0=ALU.add, op1=ALU.mult,
            )
        return xcur

    def conv_layer(bg, ly, xcur):
        """One GLU-gated causal conv layer with fp8 residual stream."""
        xnext = seq.tile([P, 2, XSTR], F8, tag="xbuf", bufs=5, name=f"xn{bg}_{ly}")
        if ly < NL - 1:
            nc.gpsimd.memset(xnext[:, :, 0:PAD], 0.0)
        for pair in range(2):
            oc_a, oc_b = pair, 2 + pair
            pb = ps.tile([P, 2, 4 * P], F32, tag="mm2", bufs=2, name=f"pb{bg}_{ly}_{pair}")
            for lt2 in range(2):
                for k in range(KW):
                    base = (ly * KW + k) * 2
                    nc.tensor.matmul(
                        pb[:, lt2, :], lhsT=cw8[:, base : base + 2, oc_b * P : (oc_b + 1) * P],
                        rhs=xcur[:, 0:2, lt2 * 4 * P + k : lt2 * 4 * P + k + 4 * P],
                        start=(k == 0), stop=(k == KW - 1), perf_mode=DRM,
                    )
            gate = wk.tile([P, 2 * 4 * P], BF16, tag="gate", bufs=4, name=f"gcv{bg}_{ly}_{pair}")
            nc.scalar.activation(gate[:], pb[:], AF.Sigmoid, bias=convb[:, ly * 4 + oc_b : ly * 4 + oc_b + 1])
            pa = ps.tile([P, 2, 4 * P], F32, tag="mm2", bufs=2, name=f"pa{bg}_{ly}_{pair}")
            for lt2 in range(2):
                for k in range(KW):
                    base = (ly * KW + k) * 2
                    nc.tensor.matmul(
                        pa[:, lt2, :], lhsT=cw8[:, base : base + 2, oc_a * P : (oc_a + 1) * P],
                        rhs=xcur[:, 0:2, lt2 * 4 * P + k : lt2 * 4 * P + k + 4 * P],
                        start=(k == 0), stop=(k == KW - 1), perf_mode=DRM,
                    )
            glu = wk.tile([P, 2 * 4 * P], BF16, tag="glu", bufs=3, name=f"glu{bg}_{ly}_{pair}")
            nc.vector.scalar_tensor_tensor(
                out=glu[:], in0=pa[:], scalar=convb[:, ly * 4 + oc_a : ly * 4 + oc_a + 1],
                in1=gate[:], op0=ALU.add, op1=ALU.mult,
            )
            nc.vector.tensor_add(
                xnext[:, pair, PAD : PAD + L], glu[:], xcur[:, pair, PAD : PAD + L]
            )
        return xnext

    def prods_emit(bg, xcur, ht8):
        prods = wk.tile([P, 2, L - 1], BF16, tag="prod", bufs=2, name=f"prod{bg}")
        for cin in range(2):
            nc.vector.tensor_mul(
                prods[:, cin, :], xcur[:, cin, PAD : PAD + L - 1], ht8[:, cin, 1:L]
            )
        return prods

    def predict_emit(bg, prods):
        osb = wk.tile([1, L - 1], F32, tag="osb", bufs=2, name=f"osb{bg}")
        for half in range(2):
            n = 4 * P if half == 0 else L - 1 - 4 * P
            zp = ps.tile([P, 4 * P], F32, tag="mm", bufs=2, name=f"zp{bg}_{half}")
            for cin in range(2):
                nc.tensor.matmul(
                    zp[0:1, :n], lhsT=ones16[:],
                    rhs=prods[:, cin, half * 4 * P : half * 4 * P + n],
                    start=(cin == 0), stop=(cin == 1),
                )
            nc.scalar.activation(osb[:, half * 4 * P : half * 4 * P + n], zp[0:1, :n], AF.Sigmoid)
        nc.sync.dma_start(out=dram["out"][bg : bg + 1, :], in_=osb[:])

    # ---------------- emission schedules ----------------
    def body_il():
        """2-deep software pipeline: front(bg) interleaved with back(bg-1)."""
        st, xst, prodst = {}, {}, {}
        prep_issue(0)
        for s in range(nb + 2):
            fb, bb, qb = s, s - 1, s - 2
            if fb < nb:
                if fb + 1 < nb:
                    prep_issue(fb + 1)
                st[fb] = prepc(fb)
            if qb >= 0:
                predict_emit(qb, prodst.pop(qb))
            if 0 <= bb < nb:
                xst[bb] = mlp(bb, st[bb][1])
            if fb < nb:
                d0 = attn_mm(fb, 0, *st[fb])
            if 0 <= bb < nb:
                xst[bb] = conv_layer(bb, 0, xst[bb])
            if fb < nb:
                attn_tp(fb, st[fb][1], d0)
                d1 = attn_mm(fb, 1, *st[fb])
            if 0 <= bb < nb:
                xst[bb] = conv_layer(bb, 1, xst[bb])
            if fb < nb:
                attn_tp(fb, st[fb][1], d1)
            if 0 <= bb < nb:
                xst[bb] = conv_layer(bb, 2, xst[bb])
                prodst[bb] = prods_emit(bb, xst.pop(bb), st[bb][1])
                st.pop(bb)

    def body_seq():
        for bg in range(nb):
            prep_issue(bg)
            lis8, ht8 = prepc(bg)
            d0 = attn_mm(bg, 0, lis8, ht8)
            attn_tp(bg, ht8, d0)
            d1 = attn_mm(bg, 1, lis8, ht8)
            attn_tp(bg, ht8, d1)
            x = mlp(bg, ht8)
            for ly in range(NL):
                x = conv_layer(bg, ly, x)
            prods = prods_emit(bg, x, ht8)
            predict_emit(bg, prods)

    sched = os.environ.get("SCHED", "il")
    bodyfn = {"il": body_il, "seq": body_seq}[sched]
    emit_weights()
    if repeat > 1:
        loop_cm = tc.For_i(0, repeat, 1)
        loop_cm.__enter__()
        bodyfn()
        loop_cm.__exit__(None, None, None)
    else:
        bodyfn()


def build(nb, repeat=1):
    nc = bacc.Bacc("TRN2", target_bir_lowering=False, debug=False)
    dram = {
        "cseq": nc.dram_tensor("cseq", [nb, L], I32, kind="ExternalInput").ap(),
        "cqc8": nc.dram_tensor("cqc8", [nb, P, 2, L], F8, kind="ExternalInput").ap(),
        "qe8": nc.dram_tensor("qe8", [nb, P, LT, D], F8, kind="ExternalInput").ap(),
        "ecpair": nc.dram_tensor("ecpair", [2, D], BF16, kind="ExternalInput").ap(),
        "onesrow": nc.dram_tensor("onesrow", [1, L], BF16, kind="ExternalInput").ap(),
        "w1t8": nc.dram_tensor("w1t8", [P, NKT, D], F8, kind="ExternalInput").ap(),
        "w1b": nc.dram_tensor("w1b", [D], F32, kind="ExternalInput").ap(),
        "w2t8": nc.dram_tensor("w2t8", [P, NKT, D], F8, kind="ExternalInput").ap(),
        "w2b": nc.dram_tensor("w2b", [D], F32, kind="ExternalInput").ap(),
        "cw8": nc.dram_tensor("cw8", [P, NL * KW * 2, 2 * D], F8, kind="ExternalInput").ap(),
        "convb": nc.dram_tensor("convb", [NL, 2 * D], F32, kind="ExternalInput").ap(),
        "out": nc.dram_tensor("out", [nb, L - 1], F32, kind="ExternalOutput").ap(),
    }
    with tile.TileContext(nc) as tc:
        with ExitStack() as ctx:
            _emit(nc, tc, ctx, dram, nb, repeat)
    nc.compile()
    return nc


_built = {}


def make_in_maps(inputs, nb):
    import ml_dtypes

    E4 = ml_dtypes.float8_e4m3fn
    BF = ml_dtypes.bfloat16
    inp = {k: np.asarray(v) for k, v in inputs.items()}
    qseq = np.ascontiguousarray(inp["question_seq"].astype(np.int32))
    cseq = np.ascontiguousarray(inp["correctness_seq"].astype(np.int32))
    # cqc8[b, p, ct, l] = cqc[b, l, ct*128+p]  (fp8)
    cqc8 = np.ascontiguousarray(
        inp["cqc_seq"].astype(np.float32)
        .transpose(0, 2, 1)
        .reshape(B, 2, P, L)
        .transpose(0, 2, 1, 3)
        .astype(E4)
    )
    ec = inp["Ec"].astype(np.float32)
    ecpair = np.ascontiguousarray(np.stack([ec[1] - ec[0], ec[0]]).astype(BF))

    # w1t8[p, kt, m] = W1_w[m, kt*128+p]
    def wt8(w):
        return np.ascontiguousarray(
            w.astype(np.float32).T.reshape(NKT, P, D).transpose(1, 0, 2).astype(E4)
        )

    # cw8[p, (ly*KW+k)*2+cin, o] = conv_w[ly, k, cin*128+p, o]
    cw8 = np.ascontiguousarray(
        inp["conv_w"].astype(np.float32)
        .reshape(NL, KW, 2, P, 2 * D)
        .transpose(3, 0, 1, 2, 4)
        .reshape(P, NL * KW * 2, 2 * D)
        .astype(E4)
    )
    # host-side embedding gather: qe8[b, p, lt, :] = Eq8[qseq[b, lt*128+p]]
    eq8_tab = inp["Eq"].astype(np.float32).astype(E4)
    qe8 = np.ascontiguousarray(
        eq8_tab[qseq].reshape(B, LT, P, D).transpose(0, 2, 1, 3)
    )
    base = {
        "ecpair": ecpair,
        "onesrow": np.ones((1, L), dtype=BF),
        "w1t8": wt8(inp["W1_w"]),
        "w1b": np.ascontiguousarray(inp["W1_b"].astype(np.float32)),
        "w2t8": wt8(inp["W2_w"]),
        "w2b": np.ascontiguousarray(inp["W2_b"].astype(np.float32)),
        "cw8": cw8,
        "convb": np.ascontiguousarray(inp["conv_b"].astype(np.float32)),
    }
    in_maps = []
    for cid in range(NCORES):
        sl = slice(cid * nb, (cid + 1) * nb)
        m = dict(base)
        m["cseq"] = cseq[sl]
        m["cqc8"] = cqc8[sl]
        m["qe8"] = qe8[sl]
        in_maps.append(m)
    return in_maps


def run_sharded(inputs, nb=B // NCORES, trace=False, **kw):
    if nb not in _built:
        _built[nb] = build(nb)
    nc = _built[nb]
    in_maps = make_in_maps(inputs, nb)
    res = run_bass_kernel_spmd(nc, in_maps, list(range(NCORES)), trace=trace, **kw)
    out = np.concatenate([res.results[c]["out"] for c in range(NCORES)], axis=0)
    return out.astype(np.float32), res


def kernel(**inputs):
    out, _ = run_sharded(inputs)
    return out


# revision 31
# speedup vs baseline: 1.0856x; 1.0856x over previous
"""Trainium2 Bass kernel for the dense_transformer problem.

Data-parallel over batch: 8 NeuronCores x (B/8) sequences each.
Heavy matmuls run in fp8e4 with DoubleRow perf mode (2 K-subtiles per
pass) accumulating in fp32 PSUM. All static per-sequence features
(embedding gathers qe/ce, their transposes, cqc) are pre-gathered,
pre-cast to fp8 and pre-laid-out on the host, so the device pipeline is
pure attention + MLP + conv. Emission is software-pipelined at
sub-stage granularity: the attention front-half of sequence bg
interleaves with the MLP/conv back-half of sequence bg-1.

ht8 kt layout: 0-1 qeT, 2-3 ceT, 4-5 cqcT (host-uploaded), 6-9 hrpT
(computed). The host permutes W1/W2 kt blocks to match.

Self-contained: only imports numpy + installed concourse package.
"""

import os
import numpy as np
from contextlib import ExitStack

import concourse.bass as bass
import concourse.bacc as bacc
import concourse.mybir as mybir
import concourse.tile as tile
from concourse.bass_utils import run_bass_kernel_spmd
from concourse.masks import make_identity, make_upper_triangular

# problem dims (hardcoded per harness contract)
B, L, D, C, NQ, KW, NL = 64, 1024, 256, 256, 10000, 4, 3
NCORES = 8
P = 128
F32 = mybir.dt.float32
BF16 = mybir.dt.bfloat16
F8 = mybir.dt.float8e4
I32 = mybir.dt.int32
AF = mybir.ActivationFunctionType
ALU = mybir.AluOpType
DRM = mybir.MatmulPerfMode.DoubleRow

LT = L // P             # 8 token tiles of 128
NKT = (4 * D + C) // P  # 10 feature tiles of H
PAD = KW - 1            # 3 causal pad cols
XSTR = 1040             # padded free stride for conv input (16-aligned)
# H feature blocks (old order qe,ce,hrp,cqc) -> new kt slots
KT_PERM = [0, 1, 2, 3, 8, 9, 4, 5, 6, 7]  # kt_new[i] gets old block KT_PERM...


def _emit(nc, tc, ctx, dram, nb, repeat=1):
    sb = ctx.enter_context(tc.tile_pool(name="sb", bufs=1))
    seq = ctx.enter_context(tc.tile_pool(name="seq", bufs=1))
    wk = ctx.enter_context(tc.tile_pool(name="wk", bufs=1))
    ps = ctx.enter_context(tc.tile_pool(name="ps", bufs=1, space="PSUM"))

    # ---------------- constants ----------------
    ident8 = sb.tile([P, P], F8, tag="ident8")
    make_identity(nc, ident8[:])
    smask8 = sb.tile([P, P], F8, tag="smask8")  # strict upper: 1.0 where part < free
    make_upper_triangular(nc, smask8[:], val=1.0, diag=False)
    ones8 = sb.tile([P, 2, 16], F8, tag="ones8")
    nc.gpsimd.memset(ones8[:], 1.0)
    ones16 = sb.tile([P, 1], BF16, tag="ones16")
    nc.gpsimd.memset(ones16[:], 1.0)
    ones32 = sb.tile([1, 1], F32, tag="ones32")
    nc.gpsimd.memset(ones32[:], 1.0)

    # biases (fp32, per-partition layout)
    w1b = sb.tile([P, 2], F32, tag="w1b")
    w2b = sb.tile([P, 2], F32, tag="w2b")
    for dh in range(2):
        nc.sync.dma_start(out=w1b[:, dh : dh + 1], in_=dram["w1b"][dh * P : (dh + 1) * P, None])
        nc.sync.dma_start(out=w2b[:, dh : dh + 1], in_=dram["w2b"][dh * P : (dh + 1) * P, None])
    convb = sb.tile([P, NL * 4], F32, tag="convb")
    for ly in range(NL):
        for oc in range(4):
            nc.sync.dma_start(
                out=convb[:, ly * 4 + oc : ly * 4 + oc + 1],
                in_=dram["convb"][ly, oc * P : (oc + 1) * P, None],
            )

    # fp8 weights (host pre-transposed/cast/permuted): single DMA each
    w1t8 = sb.tile([P, NKT, D], F8, tag="w1t8")
    w2t8 = sb.tile([P, NKT, D], F8, tag="w2t8")
    cw8 = sb.tile([P, NL * KW * 2, 2 * D], F8, tag="cw8")

    def emit_weights():
        nc.sync.dma_start(out=w1t8[:], in_=dram["w1t8"][:])
        nc.sync.dma_start(out=w2t8[:], in_=dram["w2t8"][:])
        nc.sync.dma_start(out=cw8[:], in_=dram["cw8"][:])

    # ---------------- per-sequence pipeline stages ----------------
    issued = {}

    def prep_issue(bg):
        """DMA-only: all static per-sequence features come from the host."""
        # lis8: [tok_part, lt, qe(256)|ce(256)] fp8
        lis8 = seq.tile([P, LT, 2 * D], F8, tag="lis8", bufs=3, name=f"lis8_{bg}")
        # ht8: [chan_part, kt, tok] fp8
        ht8 = seq.tile([P, NKT, L], F8, tag="ht8", bufs=3, name=f"ht8_{bg}")
        nc.sync.dma_start(out=lis8[:], in_=dram["lisq8"][bg])
        nc.sync.dma_start(out=ht8[:, 0:6, :], in_=dram["hts8"][bg])
        issued[bg] = (lis8, ht8)

    def prepc(bg):
        return issued.pop(bg)

    def attn_mm(bg, iw, lis8, ht8):
        """Scores+exp for i-half iw, then HRP accumulation; defer transposes."""
        jmax = iw * 4 + 4
        # tw8: [j_part, jb, i-col] fp8 exp-scores for this i-half
        tw8 = wk.tile([P, LT, 4 * P], F8, tag="tw8", bufs=3, name=f"tw{bg}_{iw}")
        for jb in range(jmax):
            # cols below the diagonal block are never read (jb <= ib)
            rel = max(jb * P - iw * 4 * P, 0)
            n_live = 4 * P - rel
            scp = ps.tile([P, 4 * P], F32, tag="mm", bufs=2, name=f"scp{bg}_{iw}_{jb}")
            nc.tensor.matmul(
                scp[:, 0:n_live],
                lhsT=ht8[:, 0:2, jb * P : (jb + 1) * P],
                rhs=ht8[:, 0:2, iw * 4 * P + rel : (iw + 1) * 4 * P],
                start=True, stop=True, perf_mode=DRM,
            )
            nc.scalar.activation(tw8[:, jb, rel : 4 * P], scp[:, 0:n_live], AF.Exp)
            if jb * P - iw * 4 * P >= 0:
                nc.gpsimd.tensor_mul(
                    tw8[:, jb, rel : rel + P], tw8[:, jb, rel : rel + P], smask8[:]
                )
        # row sums s[1, i] via ones-lhsT accumulation (1-col weight load),
        # then one reciprocal on the row; per-ib K=1 matmuls transpose the
        # row into the per-partition scale the Act engine needs.
        srow_ps = ps.tile([P, 4 * P], F32, tag="mm", bufs=2, name=f"srow{bg}_{iw}")
        for jb in range(jmax):
            rel = max(jb * P - iw * 4 * P, 0)
            nc.tensor.matmul(
                srow_ps[0:1, rel : 4 * P], lhsT=ones8[:, 0, 0:1],
                rhs=tw8[:, jb, rel : 4 * P],
                start=(jb == 0), stop=(jb == jmax - 1),
            )
        srow = wk.tile([1, 4 * P], F32, tag="srow", bufs=2, name=f"srowb{bg}_{iw}")
        nc.vector.tensor_scalar_add(srow[:], srow_ps[0:1, :], 1e-8)
        nc.vector.reciprocal(srow[:], srow[:])
        defer = []
        for isub in range(4):
            ib = iw * 4 + isub
            njb = ib + 1
            hrp_ps = ps.tile([P, 2 * D], F32, tag="mm", bufs=2, name=f"hrpp{bg}_{ib}")
            npair = njb // 2
            for jp in range(npair):
                js = 2 * jp
                first, last = jp == 0, (jp == npair - 1 and njb % 2 == 0)
                tsl = tw8[:, js : js + 2, isub * P : (isub + 1) * P]
                nc.tensor.matmul(
                    hrp_ps[:], lhsT=tsl, rhs=lis8[:, js : js + 2, :],
                    start=first, stop=last, perf_mode=DRM,
                )
            if njb % 2 == 1:
                tsl = tw8[:, ib, isub * P : (isub + 1) * P]
                nc.tensor.matmul(
                    hrp_ps[:], lhsT=tsl, rhs=lis8[:, ib, :],
                    start=(njb == 1), stop=True,
                )
            sp = wk.tile([P, 1], F32, tag="sp", bufs=3, name=f"sp{bg}_{ib}")
            spp = ps.tile([P, 4 * P], F32, tag="mm", bufs=2, name=f"spp{bg}_{ib}")
            nc.tensor.matmul(
                spp[:, 0:1], lhsT=srow[0:1, isub * P : (isub + 1) * P],
                rhs=ones32[0:1, 0:1], start=True, stop=True,
            )
            nc.vector.tensor_copy(sp[:], spp[:, 0:1])
            hrp8 = wk.tile([P, 2 * D], F8, tag="hrp8", bufs=3, name=f"hrp8{bg}_{ib}")
            nc.scalar.activation(hrp8[:], hrp_ps[:], AF.Identity, scale=sp[:, 0:1])
            defer.append((ib, hrp8))
        return defer

    def attn_tp(bg, ht8, defer):
        """Deferred hrpT transposes into ht8 kt 6-9."""
        for ib, hrp8 in defer:
            tp = ps.tile([P, 4, 2 * P], F8, tag="small", bufs=2, name=f"htp{bg}_{ib}")
            for dh in range(4):
                nc.tensor.transpose(
                    out=tp[:, dh, 0 : 2 * P : 2], in_=hrp8[:, dh * P : (dh + 1) * P],
                    identity=ident8[:],
                )
            nc.vector.tensor_copy(
                ht8[:, 6:10, ib * P : (ib + 1) * P], tp[:, 0:4, 0 : 2 * P : 2]
            )

    def mlp(bg, ht8):
        """H @ W1 * sigmoid(H @ W2) -> fp8 conv input xcur."""
        xcur = seq.tile([P, 2, XSTR], F8, tag="xbuf", bufs=5, name=f"xq{bg}")
        nc.gpsimd.memset(xcur[:, :, 0:PAD], 0.0)
        for dh in range(2):
            p1 = ps.tile([P, 2, 4 * P], F32, tag="mm2", bufs=2, name=f"p1_{bg}_{dh}")
            p2 = ps.tile([P, 2, 4 * P], F32, tag="mm2", bufs=2, name=f"p2_{bg}_{dh}")
            for lt2 in range(2):
                for kp in range(NKT // 2):
                    nc.tensor.matmul(
                        p2[:, lt2, :], lhsT=w2t8[:, 2 * kp : 2 * kp + 2, dh * P : (dh + 1) * P],
                        rhs=ht8[:, 2 * kp : 2 * kp + 2, lt2 * 4 * P : (lt2 + 1) * 4 * P],
                        start=(kp == 0), stop=(kp == NKT // 2 - 1), perf_mode=DRM,
                    )
                for kp in range(NKT // 2):
                    nc.tensor.matmul(
                        p1[:, lt2, :], lhsT=w1t8[:, 2 * kp : 2 * kp + 2, dh * P : (dh + 1) * P],
                        rhs=ht8[:, 2 * kp : 2 * kp + 2, lt2 * 4 * P : (lt2 + 1) * 4 * P],
                        start=(kp == 0), stop=(kp == NKT // 2 - 1), perf_mode=DRM,
                    )
            gate = wk.tile([P, 2 * 4 * P], BF16, tag="gate", bufs=4, name=f"gmlp{bg}_{dh}")
            nc.scalar.activation(gate[:], p2[:], AF.Sigmoid, bias=w2b[:, dh : dh + 1])
            nc.vector.scalar_tensor_tensor(
                out=xcur[:, dh, PAD : PAD + L],
                in0=p1[:], scalar=w1b[:, dh : dh + 1], in1=gate[:],
                op0=ALU.add, op1=ALU.mult,
            )
        return xcur

    def conv_layer(bg, ly, xcur):
        """One GLU-gated causal conv layer with fp8 residual stream."""
        xnext = seq.tile([P, 2, XSTR], F8, tag="xbuf", bufs=5, name=f"xn{bg}_{ly}")
        if ly < NL - 1:
            nc.gpsimd.memset(xnext[:, :, 0:PAD], 0.0)
        for pair in range(2):
            oc_a, oc_b = pair, 2 + pair
            pb = ps.tile([P, 2, 4 * P], F32, tag="mm2", bufs=2, name=f"pb{bg}_{ly}_{pair}")
            for lt2 in range(2):
                for k in range(KW):
                    base = (ly * KW + k) * 2
                    nc.tensor.matmul(
                        pb[:, lt2, :], lhsT=cw8[:, base : base + 2, oc_b * P : (oc_b + 1) * P],
                        rhs=xcur[:, 0:2, lt2 * 4 * P + k : lt2 * 4 * P + k + 4 * P],
                        start=(k == 0), stop=(k == KW - 1), perf_mode=DRM,
                    )
            gate = wk.tile([P, 2 * 4 * P], BF16, tag="gate", bufs=4, name=f"gcv{bg}_{ly}_{pair}")
            nc.scalar.activation(gate[:], pb[:], AF.Sigmoid, bias=convb[:, ly * 4 + oc_b : ly * 4 + oc_b + 1])
            pa = ps.tile([P, 2, 4 * P], F32, tag="mm2", bufs=2, name=f"pa{bg}_{ly}_{pair}")
            for lt2 in range(2):
                for k in range(KW):
                    base = (ly * KW + k) * 2
                    nc.tensor.matmul(
                        pa[:, lt2, :], lhsT=cw8[:, base : base + 2, oc_a * P : (oc_a + 1) * P],
                        rhs=xcur[:, 0:2, lt2 * 4 * P + k : lt2 * 4 * P + k + 4 * P],
                        start=(k == 0), stop=(k == KW - 1), perf_mode=DRM,
                    )
            glu = wk.tile([P, 2 * 4 * P], BF16, tag="glu", bufs=3, name=f"glu{bg}_{ly}_{pair}")
            nc.vector.scalar_tensor_tensor(
                out=glu[:], in0=pa[:], scalar=convb[:, ly * 4 + oc_a : ly * 4 + oc_a + 1],
                in1=gate[:], op0=ALU.add, op1=ALU.mult,
            )
            nc.gpsimd.tensor_add(
                xnext[:, pair, PAD : PAD + L], glu[:], xcur[:, pair, PAD : PAD + L]
            )
        return xnext

    def prods_emit(bg, xcur, ht8):
        prods = wk.tile([P, 2, L - 1], BF16, tag="prod", bufs=2, name=f"prod{bg}")
        for cin in range(2):
            nc.gpsimd.tensor_mul(
                prods[:, cin, :], xcur[:, cin, PAD : PAD + L - 1], ht8[:, cin, 1:L]
            )
        return prods

    def predict_emit(bg, prods):
        osb = wk.tile([1, L - 1], F32, tag="osb", bufs=2, name=f"osb{bg}")
        for half in range(2):
            n = 4 * P if half == 0 else L - 1 - 4 * P
            zp = ps.tile([P, 4 * P], F32, tag="mm", bufs=2, name=f"zp{bg}_{half}")
            for cin in range(2):
                nc.tensor.matmul(
                    zp[0:1, :n], lhsT=ones16[:],
                    rhs=prods[:, cin, half * 4 * P : half * 4 * P + n],
                    start=(cin == 0), stop=(cin == 1),
                )
            nc.scalar.activation(osb[:, half * 4 * P : half * 4 * P + n], zp[0:1, :n], AF.Sigmoid)
        nc.sync.dma_start(out=dram["out"][bg : bg + 1, :], in_=osb[:])

    # ---------------- emission schedules ----------------
    def body_il():
        """2-deep software pipeline: front(bg) interleaved with back(bg-1)."""
        st, xst, prodst = {}, {}, {}
        prep_issue(0)
        for s in range(nb + 2):
            fb, bb, qb = s, s - 1, s - 2
            if fb < nb:
                if fb + 1 < nb:
                    prep_issue(fb + 1)
                st[fb] = prepc(fb)
            if qb >= 0:
                predict_emit(qb, prodst.pop(qb))
            if 0 <= bb < nb:
                xst[bb] = mlp(bb, st[bb][1])
            if fb < nb:
                d0 = attn_mm(fb, 0, *st[fb])
            if 0 <= bb < nb:
                xst[bb] = conv_layer(bb, 0, xst[bb])
            if fb < nb:
                attn_tp(fb, st[fb][1], d0)
                d1 = attn_mm(fb, 1, *st[fb])
            if 0 <= bb < nb:
                xst[bb] = conv_layer(bb, 1, xst[bb])
            if fb < nb:
                attn_tp(fb, st[fb][1], d1)
            if 0 <= bb < nb:
                xst[bb] = conv_layer(bb, 2, xst[bb])
                prodst[bb] = prods_emit(bb, xst.pop(bb), st[bb][1])
                st.pop(bb)

    def body_seq():
        for bg in range(nb):
            prep_issue(bg)
            lis8, ht8 = prepc(bg)
            d0 = attn_mm(bg, 0, lis8, ht8)
            attn_tp(bg, ht8, d0)
            d1 = attn_mm(bg, 1, lis8, ht8)
            attn_tp(bg, ht8, d1)
            x = mlp(bg, ht8)
            for ly in range(NL):
                x = conv_layer(bg, ly, x)
            prods = prods_emit(bg, x, ht8)
            predict_emit(bg, prods)

    sched = os.environ.get("SCHED", "il")
    passes = int(os.environ.get("PASSES", "1"))
    bodyfn = {"il": body_il, "seq": body_seq}[sched]
    emit_weights()
    if repeat > 1:
        loop_cm = tc.For_i(0, repeat, 1)
        loop_cm.__enter__()
        for _ in range(passes):
            bodyfn()
        loop_cm.__exit__(None, None, None)
    else:
        bodyfn()


def build(nb, repeat=1):
    nc = bacc.Bacc("TRN2", target_bir_lowering=False, debug=False)
    dram = {
        "lisq8": nc.dram_tensor("lisq8", [nb, P, LT, 2 * D], F8, kind="ExternalInput").ap(),
        "hts8": nc.dram_tensor("hts8", [nb, P, 6, L], F8, kind="ExternalInput").ap(),
        "w1t8": nc.dram_tensor("w1t8", [P, NKT, D], F8, kind="ExternalInput").ap(),
        "w1b": nc.dram_tensor("w1b", [D], F32, kind="ExternalInput").ap(),
        "w2t8": nc.dram_tensor("w2t8", [P, NKT, D], F8, kind="ExternalInput").ap(),
        "w2b": nc.dram_tensor("w2b", [D], F32, kind="ExternalInput").ap(),
        "cw8": nc.dram_tensor("cw8", [P, NL * KW * 2, 2 * D], F8, kind="ExternalInput").ap(),
        "convb": nc.dram_tensor("convb", [NL, 2 * D], F32, kind="ExternalInput").ap(),
        "out": nc.dram_tensor("out", [nb, L - 1], F32, kind="ExternalOutput").ap(),
    }
    with tile.TileContext(nc) as tc:
        with ExitStack() as ctx:
            _emit(nc, tc, ctx, dram, nb, repeat)
    nc.compile()
    return nc


_built = {}


def make_in_maps(inputs, nb):
    import ml_dtypes

    E4 = ml_dtypes.float8_e4m3fn
    inp = {k: np.asarray(v) for k, v in inputs.items()}
    qseq = inp["question_seq"].astype(np.int64)
    cseq = inp["correctness_seq"].astype(np.int64)

    # fp8 embedding tables
    eq8 = inp["Eq"].astype(np.float32).astype(E4)
    ec8 = inp["Ec"].astype(np.float32).astype(E4)
    qe = eq8[qseq]  # [B, L, D] fp8
    ce = ec8[cseq]  # [B, L, D] fp8

    # lisq8[b, p, lt, 0:256]=qe, [256:512]=ce  (token-major partitions)
    lis = np.concatenate([qe, ce], axis=2).reshape(B, LT, P, 2 * D)
    lisq8 = np.ascontiguousarray(lis.transpose(0, 2, 1, 3))

    # hts8[b, p, kt, l]: kt 0-1 qeT, 2-3 ceT, 4-5 cqcT  (channel-major)
    qeT = qe.transpose(0, 2, 1).reshape(B, 2, P, L)      # [B, ct, p, l]
    ceT = ce.transpose(0, 2, 1).reshape(B, 2, P, L)
    cqcT = (
        inp["cqc_seq"].astype(np.float32).transpose(0, 2, 1).reshape(B, 2, P, L).astype(E4)
    )
    hts8 = np.ascontiguousarray(
        np.stack(
            [qeT[:, 0], qeT[:, 1], ceT[:, 0], ceT[:, 1], cqcT[:, 0], cqcT[:, 1]], axis=2
        )
    )  # [B, P, 6, L]

    # w1t8[p, kt_new, m] = W[m, ktold*128+p] with H-block permutation
    def wt8(w):
        t = w.astype(np.float32).T.reshape(NKT, P, D)[KT_PERM]
        return np.ascontiguousarray(t.transpose(1, 0, 2).astype(E4))

    cw8 = np.ascontiguousarray(
        inp["conv_w"].astype(np.float32)
        .reshape(NL, KW, 2, P, 2 * D)
        .transpose(3, 0, 1, 2, 4)
        .reshape(P, NL * KW * 2, 2 * D)
        .astype(E4)
    )
    base = {
        "w1t8": wt8(inp["W1_w"]),
        "w1b": np.ascontiguousarray(inp["W1_b"].astype(np.float32)),
        "w2t8": wt8(inp["W2_w"]),
        "w2b": np.ascontiguousarray(inp["W2_b"].astype(np.float32)),
        "cw8": cw8,
        "convb": np.ascontiguousarray(inp["conv_b"].astype(np.float32)),
    }
    in_maps = []
    for cid in range(NCORES):
        sl = slice(cid * nb, (cid + 1) * nb)
        m = dict(base)
        m["lisq8"] = lisq8[sl]
        m["hts8"] = hts8[sl]
        in_maps.append(m)
    return in_maps


def run_sharded(inputs, nb=B // NCORES, trace=False, **kw):
    if nb not in _built:
        _built[nb] = build(nb)
    nc = _built[nb]
    in_maps = make_in_maps(inputs, nb)
    res = run_bass_kernel_spmd(nc, in_maps, list(range(NCORES)), trace=trace, **kw)
    out = np.concatenate([res.results[c]["out"] for c in range(NCORES)], axis=0)
    return out.astype(np.float32), res


def kernel(**inputs):
    out, _ = run_sharded(inputs)
    return out
